# revision 20
# baseline (speedup 1.0000x reference)
"""Causal self-attention (B=2, T=2048, C=768, H=12) on 8 TRN2 NeuronCores.

Sharding: core c -> batch b = c//4, head-group g = c%4 (heads 3g..3g+2).
Each core computes QKV for its 3 heads, causal attention, and a partial
c_proj (its heads' rows of W_proj). Host sums the 4 partials per batch.

Device layout is fully transposed (feature dim on partitions):
  xT [768, 2048], qkv^T tiles [128, 2048], scores S^T [k, q], y^T, out^T.
Softmax over k (= partition dim of S^T) uses an appended ones-column on V:
the PV matmul then yields [y_unnorm^T; denom] in one accumulation group.
No max-subtraction: scores are ~N(0,1) (|s| < ~7), exp is fp32-safe.

All matmuls run on bf16 operands (weight loads hide fully; ~221ns per
512-wide matmul) except the V^T transposes which stay f32r. PSUM stays
fp32 throughout, so only input rounding costs accuracy (~4e-3 rel).

The attention inner loop is ACT(exp)-bound, so emission software-
pipelines across engines AND phases: S of pair p+1 before PV of pair p,
and QKV chunks for later heads plus c_proj chunks are interleaved into
the attention stream so PE has dense work while ACT runs the exps.

qkv m-tile packing (host must match):
  m0: [V_h0 | V_h1] f32r   m1: [V_h2 | -] f32r
  m2: [Q_h0 | Q_h1] bf16   m3: [K_h0 | K_h1] bf16
  m4: [Q_h2 | -] bf16      m5: [K_h2 | -] bf16
Q_h and K_h of each head sit at the same base partition (matmul
requires equal lhsT/rhs base partitions); V tiles come first so the
PE transposes can run early.
"""

import numpy as np
import ml_dtypes

import concourse.bass as bass
import concourse.mybir as mybir
import concourse.tile as tile
from concourse import bacc
from concourse.bass_utils import run_bass_kernel_spmd
from concourse.masks import make_identity, make_upper_triangular

F32 = mybir.dt.float32
F32R = mybir.dt.float32r
BF16 = mybir.dt.bfloat16
AF = mybir.ActivationFunctionType

T = 2048           # sequence length
C = 768            # embed dim
HPC = 3            # heads per core
D = 64             # head dim
W576 = HPC * 3 * D  # 576 qkv cols per core
QC = 512           # q-chunk (psum bank width in fp32)
KT = 128           # k-tile
NKT = T // KT      # 16
NQC = T // QC      # 4
NCH = C // 128     # 6 contraction chunks for qkv
SCALE = 1.0 / 8.0  # 1/sqrt(64)
# six m-tiles, all padded to 128 cols (zero weight cols in the pad)
M_W = [128, 128, 128, 128, 128, 128]
M_OFF = [0, 128, 256, 384, 512, 640]

_CACHE = {}
LAST_RESULTS = None
_TCNT = [0]


def mk_persist(pool, shape, dtype, name=None):
    if name is None:
        _TCNT[0] += 1
        name = f"pt{_TCNT[0]}"
    return pool.tile(shape, dtype, name=name, tag=name)


def build():
    nc = bacc.Bacc("TRN2", target_bir_lowering=False)

    xTb = nc.dram_tensor("xTb", [C, T], BF16, kind="ExternalInput")
    wqkvb = nc.dram_tensor("wqkvb", [C, 768], BF16, kind="ExternalInput")
    battn = nc.dram_tensor("battn", [128, 6], F32, kind="ExternalInput")
    wproj = nc.dram_tensor("wproj", [256, C], BF16, kind="ExternalInput")
    yzero = nc.dram_tensor("yzero", [64, T], BF16, kind="ExternalInput")
    yT = nc.dram_tensor("yT", [C, T], F32, kind="ExternalOutput")

    with tile.TileContext(nc) as tc, \
            tc.tile_pool(name="persist", bufs=1) as pp:
        # ---- persistent SBUF tensors ----
        ident_s = mk_persist(pp, [128, 128], F32)
        trimask_s = mk_persist(pp, [128, 128], F32)  # [k, q] = 1.0 iff k <= q
        make_identity(nc, ident_s[:, :])
        make_upper_triangular(nc, trimask_s[:, :], val=1.0, diag=True)
        ident = mk_persist(pp, [128, 128], F32R)
        trimask = mk_persist(pp, [128, 128], BF16)
        nc.vector.tensor_copy(ident[:, :], ident_s[:, :])
        nc.vector.tensor_copy(trimask[:, :], trimask_s[:, :])

        xsbb = mk_persist(pp, [128, NCH, T], BF16)   # x^T bf16
        wqkv_b = mk_persist(pp, [128, NCH, 768], BF16)
        battn_sb = mk_persist(pp, [128, 6], F32)
        wproj_sb0 = mk_persist(pp, [128, C], BF16)   # wproj rows 0:128
        wproj_sb1 = mk_persist(pp, [128, C], BF16)   # rows 128:256 (192+ = 0)
        v01 = mk_persist(pp, [128, T], F32R)  # m0: [V_h0 | V_h1]
        v2 = mk_persist(pp, [128, T], F32R)   # m1: [V_h2 | -]
        qA = mk_persist(pp, [128, T], F32R)   # m2: [Q_h0 | Q_h1]
        kA = mk_persist(pp, [128, T], F32R)   # m3: [K_h0 | K_h1]
        q2 = mk_persist(pp, [128, T], F32R)   # m4: [Q_h2 | -]
        k2 = mk_persist(pp, [128, T], F32R)   # m5: [K_h2 | -]
        vaug = mk_persist(pp, [128, NKT * HPC, 65], BF16)  # V tiles + ones col
        yA = mk_persist(pp, [128, T], BF16)   # y^T heads 0,1
        yB = mk_persist(pp, [128, T], BF16)   # y^T head 2 + zero pad

        # m0 weight columns + x t-chunk 0 first so the first QKV matmul
        # starts as early as possible; late-needed tensors load last.
        for cc in range(NCH):
            nc.sync.dma_start(wqkv_b[:, cc, 0:128], wqkvb[cc * 128:(cc + 1) * 128, 0:128])
        for cc in range(NCH):
            nc.sync.dma_start(
                xsbb[:, cc, 0:QC], xTb[cc * 128:(cc + 1) * 128, 0:QC])
        nc.sync.dma_start(battn_sb[:, :], battn[:, :])
        for cc in range(NCH):
            nc.sync.dma_start(wqkv_b[:, cc, 128:768], wqkvb[cc * 128:(cc + 1) * 128, 128:768])
        for t in range(1, NQC):
            for cc in range(NCH):
                nc.sync.dma_start(
                    xsbb[:, cc, t * QC:(t + 1) * QC],
                    xTb[cc * 128:(cc + 1) * 128, t * QC:(t + 1) * QC])
        nc.sync.dma_start(wproj_sb0[:, :], wproj[0:128, :])
        nc.sync.dma_start(wproj_sb1[:, :], wproj[128:256, :])
        nc.sync.dma_start(yB[64:128, :], yzero[:, :])

        qkv_dest = [v01, v2, qA, kA, q2, k2]
        # per head: (Q tile, K tile, base row); V^T source (tile, base row)
        qk_of = [(qA, kA, 0), (qA, kA, 64), (q2, k2, 0)]
        vt_of = [(v01, 0), (v01, 64), (v2, 0)]

        with (
            tc.tile_pool(name="psA", bufs=3, space="PSUM") as psA,
            tc.tile_pool(name="psY", bufs=2, space="PSUM") as psY,
            tc.tile_pool(name="sb", bufs=8) as sbp,
        ):
            # single strided memset for every vaug ones-column
            nc.vector.memset(vaug[:, :, 64:65], 1.0)

            def emit_qkv(m, t):
                M = 128
                dest = qkv_dest[m]
                ps = psA.tile([128, 2 * QC], F32, tag="ps", name="ps")
                for cc in range(NCH):
                    nc.tensor.matmul(
                        ps[:M, 0:QC],
                        lhsT=wqkv_b[:, cc, M_OFF[m]:M_OFF[m] + M],
                        rhs=xsbb[:, cc, t * QC:(t + 1) * QC],
                        start=(cc == 0), stop=(cc == NCH - 1),
                    )
                nc.vector.tensor_scalar_add(
                    dest[:M, t * QC:(t + 1) * QC], ps[:M, 0:QC],
                    battn_sb[:M, m:m + 1],
                )

            def emit_vtrans(h, kt4):
                vsrc, vb = vt_of[h]
                pt = psA.tile([128, 2 * QC], F32, tag="ps", name="ps")
                for j in range(4):
                    kt = kt4 * 4 + j
                    nc.tensor.transpose(
                        pt[:, j * 64:(j + 1) * 64].bitcast(F32R),
                        vsrc[vb:vb + 64, kt * KT:(kt + 1) * KT],
                        ident[vb:vb + 64, vb:vb + 64],
                    )
                vi = h * NKT + kt4 * 4
                nc.vector.tensor_copy(
                    vaug[:, vi:vi + 4, 0:64],
                    pt[:, 0:256].rearrange("p (a b) -> p a b", b=64),
                )

            def emit_attn(h, t):
                qt, kt_t, qb = qk_of[h]
                ydest, yrow = (yA, 0) if h == 0 else (yA, 64) if h == 1 else (yB, 0)
                qlo_g = t * QC
                py = psY.tile([128, QC], F32, tag="py", name="py")
                n_k = 4 * (t + 1)
                n_pair = n_k // 2

                def qlo_of(kt):
                    dm = kt - 4 * t
                    return 128 * dm if dm >= 0 else 0

                def emit_S(p):
                    ps = psA.tile([128, 2 * QC], F32, tag="ps", name="ps")
                    pT = sbp.tile([128, 2 * QC], BF16, tag="pT", name="pT")
                    for half in range(2):
                        kt = 2 * p + half
                        qlo = qlo_of(kt)
                        nc.tensor.matmul(
                            ps[:, half * QC + qlo:(half + 1) * QC],
                            lhsT=kt_t[qb:qb + 64, kt * KT:(kt + 1) * KT],
                            rhs=qt[qb:qb + 64, qlo_g + qlo:qlo_g + QC],
                            start=True, stop=True,
                        )
                    lo = qlo_of(2 * p)
                    nc.scalar.activation(
                        pT[:, lo:2 * QC], ps[:, lo:2 * QC], AF.Exp,
                        scale=SCALE,
                    )
                    for half in range(2):
                        kt = 2 * p + half
                        if kt - 4 * t >= 0:
                            o = half * QC + qlo_of(kt)
                            nc.vector.tensor_mul(
                                pT[:, o:o + 128], pT[:, o:o + 128],
                                trimask[:, :],
                            )
                    return pT

                def emit_PV(p, pT):
                    for half in range(2):
                        kt = 2 * p + half
                        qlo = qlo_of(kt)
                        nc.tensor.matmul(
                            py[0:65, qlo:QC],
                            lhsT=vaug[:, h * NKT + kt, :],
                            rhs=pT[:, half * QC + qlo:(half + 1) * QC],
                            start=(kt == 0), stop=(kt == n_k - 1),
                        )

                pTs = {0: emit_S(0)}
                for p in range(n_pair):
                    if p + 1 < n_pair:
                        pTs[p + 1] = emit_S(p + 1)
                    emit_PV(p, pTs.pop(p))

                # broadcast denom row first, then reciprocal on 64 lanes
                den = sbp.tile([1, QC], F32, tag="den", name="den")
                nc.vector.tensor_copy(den[0:1, :], py[64:65, :])
                bcast = sbp.tile([64, QC], F32, tag="bcast", name="bcast")
                nc.gpsimd.partition_broadcast(bcast[:, :], den[0:1, :])
                rec = sbp.tile([64, QC], F32, tag="rec", name="rec")
                nc.vector.reciprocal(rec[:, :], bcast[:, :])
                nc.vector.tensor_mul(
                    ydest[yrow:yrow + 64, qlo_g:qlo_g + QC],
                    py[0:64, :], rec[:, :],
                )

            def emit_proj(ct, t):
                ps = psA.tile([128, 2 * QC], F32, tag="ps", name="ps")
                nc.tensor.matmul(
                    ps[:, 0:QC],
                    lhsT=wproj_sb0[:, ct * 128:(ct + 1) * 128],
                    rhs=yA[:, t * QC:(t + 1) * QC],
                    start=True, stop=False,
                )
                nc.tensor.matmul(
                    ps[:, 0:QC],
                    lhsT=wproj_sb1[:, ct * 128:(ct + 1) * 128],
                    rhs=yB[:, t * QC:(t + 1) * QC],
                    start=False, stop=True,
                )
                osb = sbp.tile([128, QC], F32, tag="osb", name="osb")
                nc.scalar.activation(osb[:, :], ps[:, 0:QC], AF.Copy)
                nc.sync.dma_start(
                    yT[ct * 128:(ct + 1) * 128, t * QC:(t + 1) * QC],
                    osb[:, :],
                )

            # ---- schedule ----
            for t in range(NQC):
                emit_qkv(0, t)            # V01
            for t in range(NQC):
                emit_qkv(1, t)            # V2
            for h in range(HPC):
                for kt4 in range(NKT // 4):
                    emit_vtrans(h, kt4)
            for t in range(NQC):
                emit_qkv(2, t)            # Q01
            for t in range(NQC):
                emit_qkv(3, t)            # K01
            for t in range(NQC):          # attn h0 + m4 (Q2) interleaved
                emit_attn(0, t)
                emit_qkv(4, t)
            for t in range(NQC):          # attn h1 + m5 (K2) interleaved
                emit_attn(1, t)
                emit_qkv(5, t)
            for t in range(NQC):          # attn h2 + proj interleaved
                emit_attn(2, t)
                for ct in range(NCH):
                    emit_proj(ct, t)

    nc.finalize()
    return nc


def kernel(x, W_attn, b_attn, W_proj, b_proj):
    global LAST_RESULTS
    B = x.shape[0]
    x = np.asarray(x, np.float32)
    W_attn = np.asarray(W_attn, np.float32)
    b_attn = np.asarray(b_attn, np.float32)
    W_proj = np.asarray(W_proj, np.float32)
    b_proj = np.asarray(b_proj, np.float32)

    if "nc" not in _CACHE:
        _CACHE["nc"] = build()
    nc = _CACHE["nc"]

    in_maps = []
    for c in range(8):
        b, g = divmod(c, 4)
        heads = [3 * g + i for i in range(HPC)]
        h0, h1, h2 = heads
        Q = lambda h: W_attn[:, 64 * h:64 * h + 64]
        K = lambda h: W_attn[:, C + 64 * h:C + 64 * h + 64]
        V = lambda h: W_attn[:, 2 * C + 64 * h:2 * C + 64 * h + 64]
        bQ = lambda h: b_attn[64 * h:64 * h + 64]
        bK = lambda h: b_attn[C + 64 * h:C + 64 * h + 64]
        bV = lambda h: b_attn[2 * C + 64 * h:2 * C + 64 * h + 64]
        # m-tiles: [V0|V1], [V2], [Q0|Q1], [K0|K1], [Q2], [K2]
        z64 = np.zeros((C, 64), np.float32)
        wqkvb = np.ascontiguousarray(np.concatenate(
            [V(h0), V(h1), V(h2), z64, Q(h0), Q(h1), K(h0), K(h1),
             Q(h2), z64, K(h2), z64], 1)).astype(ml_dtypes.bfloat16)
        bcols = [bV(h0), bV(h1), bV(h2), np.zeros(64, np.float32),
                 bQ(h0), bQ(h1), bK(h0), bK(h1),
                 bQ(h2), np.zeros(64, np.float32),
                 bK(h2), np.zeros(64, np.float32)]
        bvec = np.concatenate(bcols)                     # [768] = 6 x 128
        battn = np.ascontiguousarray(bvec.reshape(6, 128).T)  # [128, 6]
        wproj = np.zeros((256, C), np.float32)
        wproj[:192] = np.concatenate(
            [W_proj[64 * h:64 * h + 64, :] for h in heads], 0)
        xt = np.ascontiguousarray(x[b].T)
        in_maps.append({
            "xTb": xt.astype(ml_dtypes.bfloat16),
            "wqkvb": wqkvb,
            "battn": battn,
            "wproj": wproj.astype(ml_dtypes.bfloat16),
            "yzero": np.zeros((64, T), ml_dtypes.bfloat16),
        })

    res = run_bass_kernel_spmd(nc, in_maps, core_ids=list(range(8)))
    LAST_RESULTS = res

    out = np.zeros((B, T, C), np.float32)
    for c in range(8):
        b = c // 4
        out[b] += res.results[c]["yT"].T
    out += b_proj
    return out


# revision 21
# speedup vs baseline: 1.1166x; 1.1166x over previous
"""Causal self-attention (B=2, T=2048, C=768, H=12) on 8 TRN2 NeuronCores.

Sharding: core c -> batch b = c//4, head-group g = c%4 (heads 3g..3g+2).
Each core computes QKV for its 3 heads, causal attention, and a partial
c_proj (its heads' rows of W_proj). Host sums the 4 partials per batch.

Device layout is fully transposed (feature dim on partitions):
  xT [768, 2048], qkv^T tiles [128, 2048], scores S^T [k, q], y^T, out^T.
Softmax over k (= partition dim of S^T) uses an appended ones-column on V:
the PV matmul then yields [y_unnorm^T; denom] in one accumulation group.
No max-subtraction: scores are ~N(0,1) (|s| < ~7), exp is fp32-safe.

All matmuls run on bf16 operands (weight loads hide fully; ~221ns per
512-wide matmul) except the V^T transposes which stay f32r. PSUM stays
fp32 throughout, so only input rounding costs accuracy (~4e-3 rel).

The attention inner loop is ACT(exp)-bound, so emission software-
pipelines across engines AND phases: S of pair p+1 before PV of pair p,
and QKV chunks for later heads plus c_proj chunks are interleaved into
the attention stream so PE has dense work while ACT runs the exps.

qkv m-tile packing (host must match):
  m0: [V_h0 | V_h1] f32r   m1: [V_h2 | -] f32r
  m2: [Q_h0 | Q_h1] bf16   m3: [K_h0 | K_h1] bf16
  m4: [Q_h2 | -] bf16      m5: [K_h2 | -] bf16
Q_h and K_h of each head sit at the same base partition (matmul
requires equal lhsT/rhs base partitions); V tiles come first so the
PE transposes can run early.
"""

import numpy as np
import ml_dtypes

import concourse.bass as bass
import concourse.mybir as mybir
import concourse.tile as tile
from concourse import bacc
from concourse.bass_utils import run_bass_kernel_spmd
from concourse.masks import make_identity, make_upper_triangular

F32 = mybir.dt.float32
F32R = mybir.dt.float32r
BF16 = mybir.dt.bfloat16
AF = mybir.ActivationFunctionType

T = 2048           # sequence length
C = 768            # embed dim
HPC = 3            # heads per core
D = 64             # head dim
W576 = HPC * 3 * D  # 576 qkv cols per core
QC = 512           # q-chunk (psum bank width in fp32)
KT = 128           # k-tile
NKT = T // KT      # 16
NQC = T // QC      # 4
NCH = C // 128     # 6 contraction chunks for qkv
SCALE = 1.0 / 8.0  # 1/sqrt(64)
# six m-tiles, all padded to 128 cols (zero weight cols in the pad)
M_W = [128, 128, 128, 128, 128, 128]
M_OFF = [0, 128, 256, 384, 512, 640]

_CACHE = {}
LAST_RESULTS = None
_TCNT = [0]


def mk_persist(pool, shape, dtype, name=None):
    if name is None:
        _TCNT[0] += 1
        name = f"pt{_TCNT[0]}"
    return pool.tile(shape, dtype, name=name, tag=name)


def build():
    nc = bacc.Bacc("TRN2", target_bir_lowering=False)

    xTb = nc.dram_tensor("xTb", [C, T], BF16, kind="ExternalInput")
    wqkvb = nc.dram_tensor("wqkvb", [C, 768], BF16, kind="ExternalInput")
    battn = nc.dram_tensor("battn", [128, 6], F32, kind="ExternalInput")
    wproj = nc.dram_tensor("wproj", [256, C], BF16, kind="ExternalInput")
    yzero = nc.dram_tensor("yzero", [64, T], BF16, kind="ExternalInput")
    yT = nc.dram_tensor("yT", [C, T], F32, kind="ExternalOutput")

    with tile.TileContext(nc) as tc, \
            tc.tile_pool(name="persist", bufs=1) as pp:
        # ---- persistent SBUF tensors ----
        ident_s = mk_persist(pp, [128, 128], F32)
        trimask_s = mk_persist(pp, [128, 128], F32)  # [k, q] = 1.0 iff k <= q
        make_identity(nc, ident_s[:, :])
        make_upper_triangular(nc, trimask_s[:, :], val=1.0, diag=True)
        ident = mk_persist(pp, [128, 128], F32R)
        trimask = mk_persist(pp, [128, 128], BF16)
        nc.vector.tensor_copy(ident[:, :], ident_s[:, :])
        nc.vector.tensor_copy(trimask[:, :], trimask_s[:, :])

        xsbb = mk_persist(pp, [128, NCH, T], BF16)   # x^T bf16
        wqkv_b = mk_persist(pp, [128, NCH, 768], BF16)
        battn_sb = mk_persist(pp, [128, 6], F32)
        wproj_sb0 = mk_persist(pp, [128, C], BF16)   # wproj rows 0:128
        wproj_sb1 = mk_persist(pp, [128, C], BF16)   # rows 128:256 (192+ = 0)
        v01 = mk_persist(pp, [128, T], F32R)  # m0: [V_h0 | V_h1]
        v2 = mk_persist(pp, [128, T], F32R)   # m1: [V_h2 | -]
        qA = mk_persist(pp, [128, T], F32R)   # m2: [Q_h0 | Q_h1]
        kA = mk_persist(pp, [128, T], F32R)   # m3: [K_h0 | K_h1]
        q2 = mk_persist(pp, [128, T], F32R)   # m4: [Q_h2 | -]
        k2 = mk_persist(pp, [128, T], F32R)   # m5: [K_h2 | -]
        vaug = mk_persist(pp, [128, NKT * HPC, 65], BF16)  # V tiles + ones col
        yA = mk_persist(pp, [128, T], BF16)   # y^T heads 0,1
        yB = mk_persist(pp, [128, T], BF16)   # y^T head 2 + zero pad

        # m0 weight columns + x t-chunk 0 first so the first QKV matmul
        # starts as early as possible; late-needed tensors load last.
        for cc in range(NCH):
            nc.sync.dma_start(wqkv_b[:, cc, 0:128], wqkvb[cc * 128:(cc + 1) * 128, 0:128])
        for cc in range(NCH):
            nc.sync.dma_start(
                xsbb[:, cc, 0:QC], xTb[cc * 128:(cc + 1) * 128, 0:QC])
        nc.sync.dma_start(battn_sb[:, :], battn[:, :])
        for cc in range(NCH):
            nc.sync.dma_start(wqkv_b[:, cc, 128:768], wqkvb[cc * 128:(cc + 1) * 128, 128:768])
        for t in range(1, NQC):
            for cc in range(NCH):
                nc.sync.dma_start(
                    xsbb[:, cc, t * QC:(t + 1) * QC],
                    xTb[cc * 128:(cc + 1) * 128, t * QC:(t + 1) * QC])
        nc.sync.dma_start(wproj_sb0[:, :], wproj[0:128, :])
        nc.sync.dma_start(wproj_sb1[:, :], wproj[128:256, :])
        nc.sync.dma_start(yB[64:128, :], yzero[:, :])

        qkv_dest = [v01, v2, qA, kA, q2, k2]
        # per head: (Q tile, K tile, base row); V^T source (tile, base row)
        qk_of = [(qA, kA, 0), (qA, kA, 64), (q2, k2, 0)]
        vt_of = [(v01, 0), (v01, 64), (v2, 0)]

        with (
            tc.tile_pool(name="psA", bufs=3, space="PSUM") as psA,
            tc.tile_pool(name="psY", bufs=2, space="PSUM") as psY,
            tc.tile_pool(name="sb", bufs=8) as sbp,
        ):
            # single strided memset for every vaug ones-column
            nc.vector.memset(vaug[:, :, 64:65], 1.0)

            def emit_qkv(m, t):
                M = 128
                dest = qkv_dest[m]
                ps = psA.tile([128, 2 * QC], F32, tag="ps", name="ps")
                for cc in range(NCH):
                    nc.tensor.matmul(
                        ps[:M, 0:QC],
                        lhsT=wqkv_b[:, cc, M_OFF[m]:M_OFF[m] + M],
                        rhs=xsbb[:, cc, t * QC:(t + 1) * QC],
                        start=(cc == 0), stop=(cc == NCH - 1),
                    )
                nc.vector.tensor_scalar_add(
                    dest[:M, t * QC:(t + 1) * QC], ps[:M, 0:QC],
                    battn_sb[:M, m:m + 1],
                )

            def emit_vtrans(h, kt4):
                vsrc, vb = vt_of[h]
                pt = psA.tile([128, 2 * QC], F32, tag="ps", name="ps")
                for j in range(4):
                    kt = kt4 * 4 + j
                    nc.tensor.transpose(
                        pt[:, j * 64:(j + 1) * 64].bitcast(F32R),
                        vsrc[vb:vb + 64, kt * KT:(kt + 1) * KT],
                        ident[vb:vb + 64, vb:vb + 64],
                    )
                vi = h * NKT + kt4 * 4
                nc.vector.tensor_copy(
                    vaug[:, vi:vi + 4, 0:64],
                    pt[:, 0:256].rearrange("p (a b) -> p a b", b=64),
                )

            def emit_attn(h, t):
                qt, kt_t, qb = qk_of[h]
                ydest, yrow = (yA, 0) if h == 0 else (yA, 64) if h == 1 else (yB, 0)
                qlo_g = t * QC
                py = psY.tile([128, QC], F32, tag="py", name="py")
                n_k = 4 * (t + 1)
                n_pair = n_k // 2

                def qlo_of(kt):
                    dm = kt - 4 * t
                    return 128 * dm if dm >= 0 else 0

                def emit_S(p):
                    ps = psA.tile([128, 2 * QC], F32, tag="ps", name="ps")
                    pT = sbp.tile([128, 2 * QC], BF16, tag="pT", name="pT")
                    for half in range(2):
                        kt = 2 * p + half
                        qlo = qlo_of(kt)
                        nc.tensor.matmul(
                            ps[:, half * QC + qlo:(half + 1) * QC],
                            lhsT=kt_t[qb:qb + 64, kt * KT:(kt + 1) * KT],
                            rhs=qt[qb:qb + 64, qlo_g + qlo:qlo_g + QC],
                            start=True, stop=True,
                        )
                    lo = qlo_of(2 * p)
                    nc.scalar.activation(
                        pT[:, lo:2 * QC], ps[:, lo:2 * QC], AF.Exp,
                        scale=SCALE,
                    )
                    for half in range(2):
                        kt = 2 * p + half
                        if kt - 4 * t >= 0:
                            o = half * QC + qlo_of(kt)
                            nc.vector.tensor_mul(
                                pT[:, o:o + 128], pT[:, o:o + 128],
                                trimask[:, :],
                            )
                    return pT

                def emit_PV(p, pT):
                    for half in range(2):
                        kt = 2 * p + half
                        qlo = qlo_of(kt)
                        nc.tensor.matmul(
                            py[0:65, qlo:QC],
                            lhsT=vaug[:, h * NKT + kt, :],
                            rhs=pT[:, half * QC + qlo:(half + 1) * QC],
                            start=(kt == 0), stop=(kt == n_k - 1),
                        )

                pTs = {0: emit_S(0)}
                for p in range(n_pair):
                    if p + 1 < n_pair:
                        pTs[p + 1] = emit_S(p + 1)
                    emit_PV(p, pTs.pop(p))

                # broadcast denom row first, then reciprocal on 64 lanes
                den = sbp.tile([1, QC], F32, tag="den", name="den")
                nc.vector.tensor_copy(den[0:1, :], py[64:65, :])
                bcast = sbp.tile([64, QC], F32, tag="bcast", name="bcast")
                nc.gpsimd.partition_broadcast(bcast[:, :], den[0:1, :])
                rec = sbp.tile([64, QC], F32, tag="rec", name="rec")
                nc.vector.reciprocal(rec[:, :], bcast[:, :])
                nc.vector.tensor_mul(
                    ydest[yrow:yrow + 64, qlo_g:qlo_g + QC],
                    py[0:64, :], rec[:, :],
                )

            def emit_proj(ct, t):
                ps = psA.tile([128, 2 * QC], F32, tag="ps", name="ps")
                nc.tensor.matmul(
                    ps[:, 0:QC],
                    lhsT=wproj_sb0[:, ct * 128:(ct + 1) * 128],
                    rhs=yA[:, t * QC:(t + 1) * QC],
                    start=True, stop=False,
                )
                nc.tensor.matmul(
                    ps[:, 0:QC],
                    lhsT=wproj_sb1[:, ct * 128:(ct + 1) * 128],
                    rhs=yB[:, t * QC:(t + 1) * QC],
                    start=False, stop=True,
                )
                osb = sbp.tile([128, QC], F32, tag="osb", name="osb")
                nc.scalar.activation(osb[:, :], ps[:, 0:QC], AF.Copy)
                nc.sync.dma_start(
                    yT[ct * 128:(ct + 1) * 128, t * QC:(t + 1) * QC],
                    osb[:, :],
                )

            # ---- schedule ----
            for t in range(NQC):
                emit_qkv(0, t)            # V01
            for t in range(NQC):
                emit_qkv(1, t)            # V2
            for h in range(HPC):
                for kt4 in range(NKT // 4):
                    emit_vtrans(h, kt4)
            for t in range(NQC):
                emit_qkv(2, t)            # Q01
            for t in range(NQC):
                emit_qkv(3, t)            # K01
            for t in range(NQC):          # attn h0 + m4 (Q2) interleaved
                emit_attn(0, t)
                emit_qkv(4, t)
            for t in range(NQC):          # attn h1 + m5 (K2) interleaved
                emit_attn(1, t)
                emit_qkv(5, t)
            for t in range(NQC):          # attn h2 + proj (lagged one t so
                emit_attn(2, t)           # proj never waits on the normalize
                if t > 0:                 # chain of the block just emitted)
                    for ct in range(NCH):
                        emit_proj(ct, t - 1)
            for ct in range(NCH):
                emit_proj(ct, NQC - 1)

    nc.finalize()
    return nc


def kernel(x, W_attn, b_attn, W_proj, b_proj):
    global LAST_RESULTS
    B = x.shape[0]
    x = np.asarray(x, np.float32)
    W_attn = np.asarray(W_attn, np.float32)
    b_attn = np.asarray(b_attn, np.float32)
    W_proj = np.asarray(W_proj, np.float32)
    b_proj = np.asarray(b_proj, np.float32)

    if "nc" not in _CACHE:
        _CACHE["nc"] = build()
    nc = _CACHE["nc"]

    in_maps = []
    for c in range(8):
        b, g = divmod(c, 4)
        heads = [3 * g + i for i in range(HPC)]
        h0, h1, h2 = heads
        Q = lambda h: W_attn[:, 64 * h:64 * h + 64]
        K = lambda h: W_attn[:, C + 64 * h:C + 64 * h + 64]
        V = lambda h: W_attn[:, 2 * C + 64 * h:2 * C + 64 * h + 64]
        bQ = lambda h: b_attn[64 * h:64 * h + 64]
        bK = lambda h: b_attn[C + 64 * h:C + 64 * h + 64]
        bV = lambda h: b_attn[2 * C + 64 * h:2 * C + 64 * h + 64]
        # m-tiles: [V0|V1], [V2], [Q0|Q1], [K0|K1], [Q2], [K2]
        z64 = np.zeros((C, 64), np.float32)
        wqkvb = np.ascontiguousarray(np.concatenate(
            [V(h0), V(h1), V(h2), z64, Q(h0), Q(h1), K(h0), K(h1),
             Q(h2), z64, K(h2), z64], 1)).astype(ml_dtypes.bfloat16)
        bcols = [bV(h0), bV(h1), bV(h2), np.zeros(64, np.float32),
                 bQ(h0), bQ(h1), bK(h0), bK(h1),
                 bQ(h2), np.zeros(64, np.float32),
                 bK(h2), np.zeros(64, np.float32)]
        bvec = np.concatenate(bcols)                     # [768] = 6 x 128
        battn = np.ascontiguousarray(bvec.reshape(6, 128).T)  # [128, 6]
        wproj = np.zeros((256, C), np.float32)
        wproj[:192] = np.concatenate(
            [W_proj[64 * h:64 * h + 64, :] for h in heads], 0)
        xt = np.ascontiguousarray(x[b].T)
        in_maps.append({
            "xTb": xt.astype(ml_dtypes.bfloat16),
            "wqkvb": wqkvb,
            "battn": battn,
            "wproj": wproj.astype(ml_dtypes.bfloat16),
            "yzero": np.zeros((64, T), ml_dtypes.bfloat16),
        })

    res = run_bass_kernel_spmd(nc, in_maps, core_ids=list(range(8)))
    LAST_RESULTS = res

    out = np.zeros((B, T, C), np.float32)
    for c in range(8):
        b = c // 4
        out[b] += res.results[c]["yT"].T
    out += b_proj
    return out
